# revision 67
# baseline (speedup 1.0000x reference)
"""Trainium2 Bass kernel for MultiHeadAttention (B=4, S=2048, D=1024, H=16).

Sharding: 8 cores = 4 batches x 2 sequence-halves, no collectives. Each
core computes full k/v projections for its batch and q/attention/
out-proj/LayerNorm for its half of the sequence. A host-side column
roll of x^T makes the program identical across cores (softmax over
keys is permutation-invariant): the core's q rows are always columns
[0, SQ) of its x^T.

Device program (per core), all matmuls bf16 with fp32 PSUM accumulate
(fp8e4 q/k was tried and measured: e4m3's 3-bit mantissa costs 5.5%
relative error through the peaked softmax — over the 2e-2 budget):
  qT = Wq @ xT[:, :SQ], kT = Wk @ xT  [d on partitions]
  v  = x @ Wv.T + bv  [S, D] (s on partitions), augmented with a ones
  column per head for the softmax denominator, produced in per-head-pair
  column chunks just in time for that pair's attention.

  Attention per head-pair m (heads 2m/2m+1 on partition halves of
  qt/kt tile m), per q-chunk n of 512, per key tile j of 128:
    scT[j] = kT_h . qT_h           both heads -> one [128, 2, 512] PSUM
    eT[j]  = exp(scT[j] / 8)       one wide ScalarE op -> bf16 SBUF
    av_h[:, qq, :] += eT_h[:, qq-tile]^T @ v_aug_h[j]   [128 q, 65]
  i.e. attention*V runs q-on-partitions: each matmul streams only 65
  columns (64 v dims + denominator), 8x cheaper than d-on-partitions.
  The four qq accumulators share a PSUM bank, so only the tile's first
  matmul uses start=True (a start clears has_written for the whole
  bank). Epilogue per (m, n, h): rcp[128, qq] = 1/av[:, :, 64]; per
  q-subtile multiply+cast to bf16; per (m, n, qq) a DMA-XBAR transpose
  flips [128 q, 2*64] -> aot[128 d, q] (both heads land on partition
  halves), feeding the standard out-projection.

  out = aoT.T @ Wo.T + bo; LayerNorm rstd via two tail-batched Sqrt
  activations (after the last exp, so ScalarE never swaps activation
  tables mid-stream) + DVE reciprocal.

Scheduling: PE is the bottleneck (~337us busy), ScalarE exp second
(~266us). Attention blocks run in the interleaved order (0,0), (1,0),
(0,1), (2,0), (1,1), ... so projection/out-proj filler spreads evenly
under every block; a slot-addressed unit queue emits filler after each
j's scores (a late filler input can never stall the score stream) with
v-projection in 4-key-tile quads (one PSUM-ring slot per quad so filler
never outruns the 2-buffer ring's DVE drain). Scores run one key tile
ahead of the AV rank-updates. The final attention block is split into
two 256-column halves so the t=4,5 out-projections overlap its exps.
"""

import os
import sys
from contextlib import ExitStack

for _p in ("/opt/trn_rl_repo", "/root/.axon_site/_ro/trn_rl_repo"):
    if _p not in sys.path and os.path.isdir(_p):
        sys.path.insert(0, _p)

# The kernel executes through the axon jax platform; a cpu-pinned
# JAX_PLATFORMS (used for running references) would hide the NeuronCores.
# Only safe to fix before jax is first imported.
if "jax" not in sys.modules and "axon" not in os.environ.get(
        "JAX_PLATFORMS", "axon"):
    os.environ.pop("JAX_PLATFORMS")

import ml_dtypes
import numpy as np

import concourse.bacc as bacc
import concourse.mybir as mybir
import concourse.tile as tile
from concourse import library_config
from concourse.bass_utils import run_bass_kernel_spmd

BF16 = mybir.dt.bfloat16
F32 = mybir.dt.float32
FP8 = mybir.dt.float8e4
AF = mybir.ActivationFunctionType
ALU = mybir.AluOpType
DR = mybir.MatmulPerfMode.DoubleRow

HD = 64  # head dim


def build_bass(S, SQ, D, H, dtype=BF16):
    """Build the per-core Bass program. S: kv seq len, SQ: q rows handled
    by this core, D: embed dim, H: total heads."""
    assert D == H * HD
    P = 128
    ET = D // P           # e (contraction) tiles; also head-pair count
    KP = ET // 2          # DoubleRow contraction-slab pairs
    QC = min(512, SQ)     # q free-dim chunk
    QN = SQ // QC
    KC = min(512, S)      # k-proj free-dim chunk
    KN = S // KC
    SJ = S // P           # key tiles
    TQ = SQ // P          # q row tiles
    QQ = QC // P          # q subtiles per chunk

    nc = bacc.Bacc("TRN2", debug=False)

    xT = nc.dram_tensor("xT", [D, S], dtype, kind="ExternalInput").ap()
    ws = {}
    for w in ("wq", "wk"):  # host-packed lhsT: [m, p, t*128+c]
        ws[w] = nc.dram_tensor(w, [ET, P, ET * P], dtype,
                               kind="ExternalInput").ap()
    for w in ("wv", "wo"):
        ws[w] = nc.dram_tensor(w, [D, D], dtype, kind="ExternalInput").ap()
    bs = {
        b: nc.dram_tensor(b, [D], F32, kind="ExternalInput").ap()
        for b in ("bq", "bk", "bv", "bo", "lnw", "lnb")
    }
    out = nc.dram_tensor("out", [SQ, D], dtype, kind="ExternalOutput").ap()

    with tile.TileContext(nc) as tc, ExitStack() as ctx:
        singles = ctx.enter_context(tc.tile_pool(name="singles", bufs=1))
        qkv = ctx.enter_context(tc.tile_pool(name="qkv", bufs=1))
        xp = ctx.enter_context(tc.tile_pool(name="xp", bufs=1))
        wqk = ctx.enter_context(tc.tile_pool(name="wqk", bufs=4))
        wvp = ctx.enter_context(tc.tile_pool(name="wvp", bufs=4))
        wop = ctx.enter_context(tc.tile_pool(name="wop", bufs=1))
        expp = ctx.enter_context(tc.tile_pool(name="expp", bufs=4))
        rcpp = ctx.enter_context(tc.tile_pool(name="rcpp", bufs=2))
        avsp = ctx.enter_context(tc.tile_pool(name="avsp", bufs=6))
        outp = ctx.enter_context(tc.tile_pool(name="outp", bufs=8))
        lnp = ctx.enter_context(tc.tile_pool(name="lnp", bufs=2))
        mmp = ctx.enter_context(tc.tile_pool(name="mm", bufs=2, space="PSUM"))
        scp = ctx.enter_context(tc.tile_pool(name="scp", bufs=2, space="PSUM"))
        avp = ctx.enter_context(tc.tile_pool(name="avp", bufs=2, space="PSUM"))

        nc.gpsimd.load_library(library_config.proxy)

        qt = qkv.tile([P, ET, SQ], dtype, tag="qt")
        kt = qkv.tile([P, ET, S], dtype, tag="kt")
        vt = qkv.tile([P, SJ, H, HD + 1], dtype, tag="vt")
        aot = qkv.tile([P, ET, SQ], dtype, tag="aot")

        wqk_tiles = {}

        def load_wqk(m):
            wqm = wqk.tile([P, ET, P], dtype, tag="wqk", name="wqm")
            nc.scalar.dma_start(
                wqm, ws["wq"][m].rearrange("p (t d) -> p t d", d=P))
            wkm = wqk.tile([P, ET, P], dtype, tag="wqk", name="wkm")
            nc.scalar.dma_start(
                wkm, ws["wk"][m].rearrange("p (t d) -> p t d", d=P))
            wqk_tiles[m] = (wqm, wkm)

        wv_tiles = {}

        def load_wv(m, eng=None):
            """Wv columns for head pair m: [P, ET, 2*HD]."""
            wvm = wvp.tile([P, ET, 2 * HD], dtype, tag="wv", name="wvm")
            (eng or nc.scalar).dma_start(
                wvm,
                ws["wv"].rearrange("(t p) d -> p t d", p=P)[:, :, m * P:(m + 1) * P])
            wv_tiles[m] = wvm

        # --- startup loads, interleaved across both HWDGE queues in the
        # order compute needs them (the DMA device drains roughly FIFO):
        # weights for pair 0 -> x8/x column quarters -> constants between.
        xt = xp.tile([P, ET, S], dtype, tag="xt")
        xv = xT.rearrange("(t p) s -> p t s", p=P)
        SQ4 = S // 4

        def load_x_q4(tile_, view, q4, eng):
            eng.dma_start(
                tile_[:, :, q4 * SQ4:(q4 + 1) * SQ4],
                view[:, :, q4 * SQ4:(q4 + 1) * SQ4])

        # All startup DMAs go on the SP queue in compute-need order; the
        # Activation sequencer must stay clear so the first exps issue the
        # moment their scores land.
        load_wqk(0)
        load_x_q4(xt, xv, 0, nc.sync)
        bqk = singles.tile([P, 2 * ET], F32, tag="bqk")
        nc.sync.dma_start(bqk[:, :ET], bs["bq"].rearrange("(t p) -> p t", p=P))
        nc.sync.dma_start(bqk[:, ET:], bs["bk"].rearrange("(t p) -> p t", p=P))
        load_wv(0, nc.sync)
        # free-dim bias rows, physically replicated across partitions
        # (compute engines can't read partition-step-0 APs); LN scale and
        # shift as bf16 so the tail tensor_tensors hit the DVE 2x mode
        brow = {}
        stg = singles.tile([1, D], F32, tag="stg")
        for b, bdt in (("bv", F32), ("bo", F32), ("lnw", BF16),
                       ("lnb", BF16)):
            t = singles.tile([P, D], bdt, tag=b)
            if bdt == F32:
                nc.sync.dma_start(t[0:1, :], bs[b][None, :])
            else:
                nc.sync.dma_start(stg, bs[b][None, :])
                nc.vector.tensor_copy(t[0:1, :], stg)
            nc.gpsimd.partition_broadcast(t, t[0:1, :])
            brow[b] = t
        eps = singles.tile([P, 1], F32, tag="eps")
        nc.vector.memset(eps, 1e-5)
        nc.vector.memset(vt[:, :, :, HD:HD + 1], 1.0)
        # x column quarters strictly in deadline order (the DMA device
        # drains FIFO; each quarter feeds the j-range that consumes it)
        load_x_q4(xt, xv, 1, nc.sync)
        load_wqk(1)
        load_x_q4(xt, xv, 2, nc.sync)
        load_x_q4(xt, xv, 3, nc.sync)
        load_wv(1, nc.sync)

        def q_chunk(m, n):
            """q projection chunk n of head pair m."""
            wqm = wqk_tiles[m][0]
            ps = mmp.tile([P, 512], F32, tag="mm", name="ps")[:, :QC]
            for k in range(ET):
                nc.tensor.matmul(
                    ps, wqm[:, k, :], xt[:, k, n * QC:(n + 1) * QC],
                    start=(k == 0), stop=(k == ET - 1),
                )
            nc.vector.tensor_scalar_add(
                qt[:, m, n * QC:(n + 1) * QC], ps, bqk[:, m:m + 1])

        def k_chunk(m, n):
            """k projection chunk n of head pair m."""
            wkm = wqk_tiles[m][1]
            ps = mmp.tile([P, 512], F32, tag="mm", name="ps")[:, :KC]
            for k in range(ET):
                nc.tensor.matmul(
                    ps, wkm[:, k, :], xt[:, k, n * KC:(n + 1) * KC],
                    start=(k == 0), stop=(k == ET - 1),
                )
            nc.vector.tensor_scalar_add(
                kt[:, m, n * KC:(n + 1) * KC], ps,
                bqk[:, ET + m:ET + m + 1])

        def v_quad(m, jq):
            """v projection for head pair m, s-tiles 4jq..4jq+3 — one PSUM
            ring slot and one bias op per four key tiles, so filler never
            outruns the two-buffer ring's DVE drain round-trip."""
            ps = mmp.tile([P, 512], F32, tag="mm", name="ps")
            for jj in range(4):
                j = 4 * jq + jj
                for k in range(ET):
                    nc.tensor.matmul(
                        ps[:, jj * P:(jj + 1) * P],
                        xt[:, k, j * P:(j + 1) * P], wv_tiles[m][:, k, :],
                        start=(k == 0), stop=(k == ET - 1),
                    )
            nc.vector.tensor_tensor(
                vt[:, 4 * jq:4 * jq + 4, 2 * m:2 * m + 2, 0:HD],
                ps.rearrange("p (j h d) -> p j h d", d=HD, h=2),
                brow["bv"][:, m * P:(m + 1) * P].rearrange(
                    "p (h d) -> p h d", d=HD).unsqueeze(1).broadcast_to(
                        [P, 4, 2, HD]),
                ALU.add,
            )

        def att_sc(m, j, qcol, qn):
            """score pair for (head pair m, k-tile j, q columns
            [qcol, qcol + 128*qn))."""
            qw = qn * P
            sc = scp.tile([P, 2, 512], F32, tag="sc", name="sc")
            nc.tensor.matmul(
                sc[:, 0, :qw],
                kt[0:HD, m, j * P:(j + 1) * P],
                qt[0:HD, m, qcol:qcol + qw],
            )
            nc.tensor.matmul(
                sc[:, 1, :qw],
                kt[HD:P, m, j * P:(j + 1) * P],
                qt[HD:P, m, qcol:qcol + qw],
            )
            return sc

        def att_exp(sc, qn):
            qw = qn * P
            et = expp.tile([P, 2, 512], dtype, tag="exp", name="et")
            nc.scalar.activation(et[:, :, :qw], sc[:, :, :qw], AF.Exp,
                                 scale=0.125)
            return et

        def att_av(m, j, qn, et, ava, avb):
            """rank-update of the q-on-partitions attention accumulators.

            The four qq accumulation groups share one PSUM bank, and a
            start=True matmul clears the has_written bits for the WHOLE
            2KB bank (zero-region granularity). Only the tile's very first
            matmul may use start=True; the other groups' j=0 matmuls run
            start=False — their bits are clear after that bank wipe, so
            they overwrite (fresh j=0 write) and set bits for j>=1 to
            accumulate. Interleaved start=True per group would wipe the
            sibling groups' j=0 partials."""
            for h, av in ((0, ava), (1, avb)):
                for qq in range(qn):
                    nc.tensor.matmul(
                        av[:, qq, :], et[:, h, qq * P:(qq + 1) * P],
                        vt[:, j, 2 * m + h, :],
                        start=(j == 0 and qq == 0), stop=(j == SJ - 1),
                        skip_group_check=True,
                    )

        def att_norm(m, qcol, qn, ava, avb):
            """normalize by the ones-column denominator, cast to bf16 and
            DMA-XBAR-transpose into aot (d on partitions)."""
            rcps = []
            for av in (ava, avb):
                rcp = rcpp.tile([P, 4], F32, tag="rcp", name="rcp")[:, :qn]
                nc.vector.reciprocal(rcp, av[:, :qn, HD])
                rcps.append(rcp)
            for qq in range(qn):
                avs = avsp.tile([P, 2, HD], dtype, tag="avs", name="avs")
                for h, av in ((0, ava), (1, avb)):
                    nc.vector.tensor_scalar_mul(
                        avs[:, h, :], av[:, qq, 0:HD], rcps[h][:, qq:qq + 1])
                # alternate queues: halves the in-order sequencer backlog
                eng = nc.sync if qq % 2 == 0 else nc.scalar
                eng.dma_start(
                    aot[:, m, qcol + qq * P:qcol + (qq + 1) * P],
                    avs.rearrange("p h d -> p (h d)"), transpose=True)

        def attention(m, n, q0, qn, fill):
            """q-subtiles [q0, q0+qn) of chunk n, head pair m. fill(j)
            emits filler work; it runs after j's scores/exp (so a late
            filler input can never stall the ScalarE-feeding score stream)
            but before j's AV (so same-j v tiles are topologically
            ordered)."""
            qcol = n * QC + q0 * P
            ava = avp.tile([P, 4, 65], F32, tag="av", name="av")
            avb = avp.tile([P, 4, 65], F32, tag="av", name="av")
            # scores run one key-tile ahead of the AV rank-updates so an
            # AV-side stall (e.g. the avp ring waiting on the previous
            # block's normalize) never blocks the ScalarE-feeding stream
            sc = att_sc(m, 0, qcol, qn)
            for j in range(SJ):
                et = att_exp(sc, qn)
                if j + 1 < SJ:
                    sc = att_sc(m, j + 1, qcol, qn)
                fill(j)
                att_av(m, j, qn, et, ava, avb)
            att_norm(m, qcol, qn, ava, avb)

        # LN statistics for all 8 q row tiles; the rstd sqrt runs in two
        # tail batches so ScalarE never swaps off the exp table mid-stream.
        mv8 = singles.tile([P, TQ, 2], F32, tag="mv8")
        sd8 = singles.tile([P, TQ], F32, tag="sd8")
        rstd8 = singles.tile([P, TQ], F32, tag="rstd8")
        ot_tiles = {}

        def out_ln_chunk(t, nn):
            """Out-projection chunk nn of q row tile t."""
            if nn == 0:
                ot_tiles[t] = outp.tile([P, D], dtype, tag="ot", name="ot")
            ot = ot_tiles[t]
            VC = 512
            ps = mmp.tile([P, 512], F32, tag="mm", name="ps")[:, :VC]
            for k in range(ET):
                nc.tensor.matmul(
                    ps, aot[:, k, t * P:(t + 1) * P],
                    wo[:, k, nn * VC:(nn + 1) * VC],
                    start=(k == 0), stop=(k == ET - 1),
                )
            nc.vector.tensor_tensor(
                ot[:, nn * VC:(nn + 1) * VC], ps,
                brow["bo"][:, nn * VC:(nn + 1) * VC], ALU.add)

        def out_ln_stats(t):
            """LN statistics for q row tile t."""
            FSUB = min(512, D)
            NSUB = D // FSUB
            ot = ot_tiles[t]
            scr = lnp.tile([P, NSUB * 6], F32, tag="scr", name="scr")
            stats = scr.rearrange("p (s f) -> p s f", f=6)
            otv = ot.rearrange("p (s f) -> p s f", f=FSUB)
            for sbi in range(NSUB):
                nc.vector.bn_stats(stats[:, sbi, :], otv[:, sbi, :])
            nc.vector.bn_aggr(mv8[:, t, :], scr)

        def out_ln_pre(t):
            for nn in range(2):
                out_ln_chunk(t, nn)
            out_ln_stats(t)

        def rstd_batch(t0, t1):
            """rstd for q row tiles [t0, t1) (no exps run after the
            first batch, so later batches reuse the loaded Sqrt table)."""
            nc.scalar.activation(
                sd8[:, t0:t1], mv8[:, t0:t1, 1], AF.Sqrt, bias=eps)
            nc.vector.reciprocal(rstd8[:, t0:t1], sd8[:, t0:t1])

        def out_ln_post(t, eng):
            """Apply LN and store q row tile t (all three elementwise ops
            on the given engine so two tiles can finish in parallel)."""
            ot = ot_tiles.pop(t)
            eng.tensor_scalar(
                ot, ot, mv8[:, t, 0:1], rstd8[:, t:t + 1],
                ALU.subtract, ALU.mult)
            eng.tensor_tensor(ot, ot, brow["lnw"], ALU.mult)
            eng.tensor_tensor(ot, ot, brow["lnb"], ALU.add)
            nc.sync.dma_start(
                out.rearrange("(t p) d -> p t d", p=P)[:, t, :], ot)

        # --- emission schedule ---
        # Interleaved block order spreads projection/out-proj filler into
        # the PE slack under the ScalarE-bound attention blocks.
        wo = wop.tile([P, ET, D], dtype, tag="wo")
        q_chunk(0, 0)
        k_chunk(0, 0)

        # Blocks are (m, n, q0, qn): q-subtiles [q0, q0+qn) of chunk n.
        # The final block is split in half so the t=4,5 out-projections can
        # start under the second half's exps instead of in the drain tail.
        order = [(0, 0, 0, QQ)]
        for m in range(1, ET):
            order += [(m, 0, 0, QQ), (m - 1, 1, 0, QQ)]
        order += [(ET - 1, 1, 0, QQ // 2), (ET - 1, 1, QQ // 2, QQ - QQ // 2)]

        B00 = order[0]
        # Unit scheduling: each block has 16 fill slots (one per j); a
        # unit placed at slot s is emitted at j=s. `at` pins a unit to the
        # earliest free slot >= s (deadline-ordered placement).
        slots = {blk: [[] for _ in range(SJ)] for blk in order}

        def at(blk, s, fn):
            sl = slots[blk]
            while s < SJ and len(sl[s]) >= 2:
                s += 1
            assert s < SJ, f"no free slot in {blk}"
            sl[s].append(fn)

        # v pair m: quads 0-1 (key tiles 0..7) just-in-time inside block
        # (m, 0); quads 2-3 plus next pair's first projection chunks in the
        # preceding block.
        at(B00, 0, lambda: v_quad(0, 0))
        at(B00, 1, lambda: k_chunk(0, 1))
        at(B00, 2, lambda: q_chunk(0, 1))
        at(B00, 4, lambda: v_quad(0, 1))
        at(B00, 6, lambda: k_chunk(0, 2))
        at(B00, 8, lambda: v_quad(0, 2))
        at(B00, 10, lambda: k_chunk(0, 3))
        at(B00, 12, lambda: v_quad(0, 3))
        if ET >= 2:
            at((2, 0, 0, QQ), 8, lambda: nc.sync.dma_start(
                wo, ws["wo"].rearrange("(t p) d -> p t d", p=P)))
        for m in range(1, ET):
            prev = order[order.index((m, 0, 0, QQ)) - 1]
            at((m, 0, 0, QQ), 0, lambda m=m: v_quad(m, 0))
            at((m, 0, 0, QQ), 1, lambda m=m: k_chunk(m, 1))
            at((m, 0, 0, QQ), 2, lambda m=m: q_chunk(m, 1))
            at((m, 0, 0, QQ), 4, lambda m=m: v_quad(m, 1))
            at((m, 0, 0, QQ), 6, lambda m=m: k_chunk(m, 2))
            at((m, 0, 0, QQ), 10, lambda m=m: k_chunk(m, 3))
            # m=1's prev is the crowded, DMA-bound first block: shift its
            # hosted units past the x-quarter arrival times
            base = 11 if m == 1 else 6
            at(prev, base, lambda m=m: v_quad(m, 2))
            at(prev, base + 1, lambda m=m: v_quad(m, 3))
            at(prev, base + 1, lambda m=m: k_chunk(m, 0))
            at(prev, base + 2, lambda m=m: q_chunk(m, 0))
            if m + 1 < ET:
                # weights for pair m+1, loaded one block earlier than use
                host = (m - 1, 0, 0, QQ) if m > 1 else B00
                at(host, 8, lambda m=m: load_wqk(m + 1))
                at(host, 9, lambda m=m: load_wv(m + 1))
        # out-proj/stats for t=0..3 ready after (7,0); spread over the last
        # two blocks (spaced so the PE bursts don't starve ScalarE)
        at((ET - 2, 1, 0, QQ), 0, lambda: out_ln_pre(0))
        at((ET - 2, 1, 0, QQ), 5, lambda: out_ln_pre(1))
        at((ET - 2, 1, 0, QQ), 10, lambda: out_ln_pre(2))
        at(order[-2], 0, lambda: out_ln_pre(3))
        # t=4,5 fully finish under the final half-block's exps (the extra
        # Sqrt table round-trip costs ~2.6us of idle ScalarE time but pulls
        # two LN finalizations plus their stores out of the drain tail)
        at(order[-1], 0, lambda: out_ln_chunk(4, 0))
        at(order[-1], 2, lambda: out_ln_chunk(5, 0))
        at(order[-1], 4, lambda: out_ln_chunk(4, 1))
        at(order[-1], 6, lambda: out_ln_stats(4))
        at(order[-1], 8, lambda: out_ln_chunk(5, 1))
        at(order[-1], 10, lambda: out_ln_stats(5))
        at(order[-1], 12, lambda: rstd_batch(4, 6))
        at(order[-1], 13, lambda: out_ln_post(4, nc.gpsimd))
        at(order[-1], 14, lambda: out_ln_post(5, nc.gpsimd))

        for blk in order:
            sl = slots[blk]

            def fill(j, sl=sl):
                for fn in sl[j]:
                    fn()
                sl[j] = ()
            attention(*blk, fill)
            assert all(not u for u in sl), f"unemitted units in {blk}"

        # tail: finish t=0..3 (two on DVE to free their ot bufs fast, two
        # on GpSimd in parallel), then t=4..7 with the out-proj chunks
        # t-interleaved so PSUM-ring drains pipeline instead of serialize
        rstd_batch(0, 4)
        out_ln_post(0, nc.vector)
        out_ln_post(1, nc.vector)
        out_ln_post(2, nc.vector)
        out_ln_post(3, nc.gpsimd)
        out_ln_chunk(6, 0)
        out_ln_chunk(7, 0)
        out_ln_chunk(6, 1)
        out_ln_stats(6)
        out_ln_chunk(7, 1)
        out_ln_stats(7)
        rstd_batch(6, 8)
        out_ln_post(6, nc.vector)
        out_ln_post(7, nc.vector)

    nc.compile()
    return nc


# ---------------------------------------------------------------- host side

_CACHE = {}


def _get_nc(S, SQ, D, H):
    key = (S, SQ, D, H)
    if key not in _CACHE:
        _CACHE[key] = build_bass(S, SQ, D, H)
    return _CACHE[key]


def make_in_maps(x, Wq, bq, Wk, bk, Wv, bv, Wo, bo, ln_w, ln_b, n_cores=8):
    """Shard full inputs into per-core input maps (batch x seq-half)."""
    B, S, D = x.shape
    halves = n_cores // B
    SQ = S // halves
    bf = ml_dtypes.bfloat16
    f8 = ml_dtypes.float8_e4m3
    ET = D // 128
    KP = ET // 2

    def pack_qk(W):
        # [m, p, t*128+d] = W.T[t*128+p, m*128+d]
        w4 = np.asarray(W, np.float32).T.reshape(ET, 128, ET, 128)
        return np.ascontiguousarray(
            w4.transpose(2, 1, 0, 3).reshape(ET, 128, ET * 128)).astype(bf)

    common = {
        "wq": pack_qk(Wq),
        "wk": pack_qk(Wk),
        "wv": np.ascontiguousarray(np.asarray(Wv).T).astype(bf),
        "wo": np.ascontiguousarray(np.asarray(Wo).T).astype(bf),
        "bq": np.asarray(bq, np.float32), "bk": np.asarray(bk, np.float32),
        "bv": np.asarray(bv, np.float32), "bo": np.asarray(bo, np.float32),
        "lnw": np.asarray(ln_w, np.float32), "lnb": np.asarray(ln_b, np.float32),
    }
    in_maps = []
    for c in range(n_cores):
        b, half = c // halves, c % halves
        xTb = np.asarray(x[b], np.float32).T
        if half:
            xTb = np.roll(xTb, -half * SQ, axis=1)
        in_maps.append({
            "xT": np.ascontiguousarray(xTb).astype(bf),
            **common,
        })
    return in_maps, SQ


def kernel(x, Wq, bq, Wk, bk, Wv, bv, Wo, bo, ln_w, ln_b, _trace=False):
    x = np.asarray(x)
    B, S, D = x.shape
    n_cores = 8
    in_maps, SQ = make_in_maps(x, Wq, bq, Wk, bk, Wv, bv, Wo, bo, ln_w, ln_b,
                               n_cores)
    nc = _get_nc(S, SQ, D, 16)
    res = run_bass_kernel_spmd(nc, in_maps, list(range(n_cores)), trace=_trace)
    out = np.empty((B, S, D), np.float32)
    halves = n_cores // B
    for c in range(n_cores):
        b, half = c // halves, c % halves
        out[b, half * SQ:(half + 1) * SQ] = np.asarray(
            res.results[c]["out"], np.float32)
    kernel.last_result = res
    return out


if __name__ == "__main__":
    nc = build_bass(2048, 1024, 1024, 16)
    print("built ok")
